# revision 3
# baseline (speedup 1.0000x reference)
"""Trainium2 Bass kernel for nn_DWTenhance (Haar DWT + dual MDTA + inverse DWT).

Exact algorithmic reformulation:
  Per sample the whole network is an affine map of the 2x2-polyphase
  representation P (256 x n, n = 128*128) of the image:
      out_polyphase = W_big @ P + b_big,
  where W_big = SY @ blockdiag(pw_ll A_ll (vw_ll@AN_ll), pw_h A_h (vw_h@AN_h))
  (AN/SY = Haar analysis/synthesis as 256x256 channel mixers) and the
  attention matrices A depend on the data only through the 256x256 Gram
  matrix C = P@P^T and the row sums s = P@1:
      q k^T = Wq~ C Wk~^T + (Wq~ s) kb^T + qb (Wk~ s)^T + n qb kb^T,
      |q_i|^2 = (Wq~ C Wq~^T)_ii + 2 qb_i (Wq~ s)_i + n qb_i^2, etc.

  Pass 1: stream image, PE-transpose 128x128 tiles to pixel-major layout,
          accumulate C (+ s via two ones-columns) in PSUM with f32r matmuls.
  Tiny:   on-chip small-matrix algebra + row softmax -> W_big^T (f32r) and
          bias column per sample.
  Pass 2: stream image again, apply W_big^T as a 256->256 block conv on the
          polyphase channels, add bias, interleave back, store.

Data parallel over batch: 16 samples / 8 NeuronCores = 2 samples per core.
"""
import sys

sys.path.insert(0, '/opt/trn_rl_repo')

import numpy as np

import concourse.bass as bass
import concourse.tile as tile
from concourse import bacc, mybir
from concourse.masks import make_identity

F32 = mybir.dt.float32
F32R = mybir.dt.float32r
AX = mybir.AxisListType
ALU = mybir.AluOpType
ACTF = mybir.ActivationFunctionType

B, C, H, W = 16, 64, 256, 256
NCORES = 8
SPC = B // NCORES           # samples per core
H2, W2 = H // 2, W // 2
NPIX = H2 * W2              # 16384
PC = 4 * C                  # 256 polyphase channels
R2 = 8                      # half-res rows per streamed tile
NT = H2 // R2               # tiles per sample
EPS = 1e-12

BRANCHES = (('l', C), ('h', 3 * C))


def _pidx(dx, dy, ci):
    # polyphase channel order; must match the pass-1 transpose layout
    return dx * 2 * C + dy * C + ci


def _build_AN_SY():
    AN = np.zeros((PC, PC))
    SY = np.zeros((PC, PC))
    for ci in range(C):
        a, b, c_, d = (_pidx(0, 0, ci), _pidx(1, 0, ci),
                       _pidx(0, 1, ci), _pidx(1, 1, ci))
        AN[ci, [a, b, c_, d]] = [0.5, 0.5, 0.5, 0.5]
        AN[C + 3 * ci + 0, [a, b, c_, d]] = [0.5, -0.5, 0.5, -0.5]
        AN[C + 3 * ci + 1, [a, b, c_, d]] = [0.5, 0.5, -0.5, -0.5]
        AN[C + 3 * ci + 2, [a, b, c_, d]] = [0.5, -0.5, -0.5, 0.5]
        l, h1, h2, h3 = ci, C + 3 * ci, C + 3 * ci + 1, C + 3 * ci + 2
        SY[a, [l, h1, h2, h3]] = [0.5, 0.5, 0.5, 0.5]
        SY[b, [l, h1, h2, h3]] = [0.5, -0.5, 0.5, -0.5]
        SY[c_, [l, h1, h2, h3]] = [0.5, 0.5, -0.5, -0.5]
        SY[d, [l, h1, h2, h3]] = [0.5, -0.5, -0.5, 0.5]
    return AN, SY


def _chunks(c):
    out = []
    i = 0
    while i < c:
        out.append((i, min(128, c - i)))
        i += 128
    return out


def build_host_consts(inputs):
    """float64 host preprocessing of the small weights -> DRAM const arrays."""
    AN, SY = _build_AN_SY()
    f = np.float64
    consts = {}
    temps = {}
    bias_vec = np.zeros(PC)
    for br, key, c in (('l', 'll', C), ('h', 'h', 3 * C)):
        qw, qb = inputs[f'{key}_qw'].astype(f), inputs[f'{key}_qb'].astype(f)
        kw, kb = inputs[f'{key}_kw'].astype(f), inputs[f'{key}_kb'].astype(f)
        vw, vb = inputs[f'{key}_vw'].astype(f), inputs[f'{key}_vb'].astype(f)
        pw, pb = inputs[f'{key}_pw'].astype(f), inputs[f'{key}_pb'].astype(f)
        temps[br] = float(np.asarray(inputs[f'{key}_temp']).reshape(-1)[0])
        ANb = AN[:C] if br == 'l' else AN[C:]
        SYb = SY[:, :C] if br == 'l' else SY[:, C:]
        Wq = qw @ ANb          # (c, 256)
        Wk = kw @ ANb
        VAN = vw @ ANb         # (c, 256)
        PS = SYb @ pw          # (256, c)
        chs = _chunks(c)
        mc = len(chs)
        consts[f'wqt_{br}'] = np.ascontiguousarray(
            Wq.T.reshape(2, 128, c)).astype(np.float32)
        consts[f'wkt_{br}'] = np.ascontiguousarray(
            Wk.T.reshape(2, 128, c)).astype(np.float32)
        van = np.zeros((mc, 128, PC))
        pst = np.zeros((mc, 128, PC))
        eye = np.zeros((mc, 128, c))
        vbf = np.zeros((mc, 128, c))
        for mi, (o, sz) in enumerate(chs):
            van[mi, :sz] = VAN[o:o + sz]
            pst[mi, :sz] = PS.T[o:o + sz]
            eye[mi, :sz, o:o + sz] = np.eye(sz)
            vbf[mi, :, :] = vb[None, :]
        consts[f'van_{br}'] = van.astype(np.float32)
        consts[f'pst_{br}'] = pst.astype(np.float32)
        consts[f'eye_{br}'] = eye.astype(np.float32)
        consts[f'vbf_{br}'] = vbf.astype(np.float32)

        def colchunks(v):
            arr = np.zeros((mc, 128, 1))
            for mi, (o, sz) in enumerate(chs):
                arr[mi, :sz, 0] = v[o:o + sz]
            return arr.astype(np.float32)
        consts[f'qb2_{br}'] = colchunks(2.0 * qb)
        consts[f'qb2n_{br}'] = colchunks(NPIX * qb * qb)
        consts[f'kb2_{br}'] = colchunks(2.0 * kb)
        consts[f'kb2n_{br}'] = colchunks(NPIX * kb * kb)
        consts[f'qbrow_{br}'] = qb[None, :].astype(np.float32)
        consts[f'kbrow_{br}'] = kb[None, :].astype(np.float32)
        bias_vec += SYb @ pb
    consts['bconst'] = np.ascontiguousarray(
        bias_vec.reshape(2, 128, 1)).astype(np.float32)
    return consts, temps


CSHAPES = {
    'wqt_l': [2, 128, C], 'wkt_l': [2, 128, C],
    'wqt_h': [2, 128, 3 * C], 'wkt_h': [2, 128, 3 * C],
    'van_l': [1, 128, PC], 'pst_l': [1, 128, PC],
    'van_h': [2, 128, PC], 'pst_h': [2, 128, PC],
    'eye_l': [1, 128, C], 'vbf_l': [1, 128, C],
    'eye_h': [2, 128, 3 * C], 'vbf_h': [2, 128, 3 * C],
    'qb2_l': [1, 128, 1], 'qb2n_l': [1, 128, 1],
    'kb2_l': [1, 128, 1], 'kb2n_l': [1, 128, 1],
    'qb2_h': [2, 128, 1], 'qb2n_h': [2, 128, 1],
    'kb2_h': [2, 128, 1], 'kb2n_h': [2, 128, 1],
    'qbrow_l': [1, C], 'kbrow_l': [1, C],
    'qbrow_h': [1, 3 * C], 'kbrow_h': [1, 3 * C],
    'bconst': [2, 128, 1],
}


def build_program(temps):
    """Build the Bacc program one core runs (SPC samples, full pipeline)."""
    nc = bacc.Bacc()

    imgd = nc.declare_dram_parameter('img', [SPC, C, H, W], F32, isOutput=False)
    outd = nc.declare_dram_parameter('out', [SPC, C, H, W], F32, isOutput=True)
    cdecl = {n: nc.declare_dram_parameter(n, s, F32, isOutput=False)
             for n, s in CSHAPES.items()}

    with tile.TileContext(nc) as tc:
        with tc.tile_pool(name='cst', bufs=1) as cst, \
             tc.tile_pool(name='io', bufs=3) as io, \
             tc.tile_pool(name='ptp', bufs=3) as ptp, \
             tc.tile_pool(name='work', bufs=2) as work, \
             tc.tile_pool(name='xp', bufs=2, space='PSUM') as xp, \
             tc.tile_pool(name='cps', bufs=1, space='PSUM') as cps, \
             tc.tile_pool(name='tps', bufs=1, space='PSUM') as tps:

            # ---------------- constants ----------------
            ident = cst.tile([128, 128], F32)
            make_identity(nc, ident)
            ones2 = cst.tile([128, 2], F32)
            nc.vector.memset(ones2, 1.0)
            ones_row = cst.tile([1, 128], F32)
            nc.vector.memset(ones_row, 1.0)
            csb = {}
            for name, shp in CSHAPES.items():
                if len(shp) == 3:
                    t = cst.tile([128, shp[0], shp[2]], F32, tag=name)
                    nc.sync.dma_start(
                        out=t,
                        in_=cdecl[name][:, :, :].rearrange('c p x -> p c x'))
                else:
                    t = cst.tile([1, shp[1]], F32, tag=name)
                    nc.sync.dma_start(out=t, in_=cdecl[name][:, :])
                csb[name] = t

            def cget(name):
                return csb[name]

            w_sb = {}
            bcol_sb = {}

            # =============== PASS 1: Gram accumulation ===============
            def pass1(s):
                cpsums = [cps.tile([128, 258], F32, tag=f'c{rc}', name=f'cp{rc}')
                          for rc in range(2)]
                for ti in range(NT):
                    r0 = 2 * R2 * ti
                    img_sb = io.tile([128, R2, 256], F32, tag='in1',
                                     name='img1')
                    for dy in range(2):
                        nc.sync.dma_start(
                            out=img_sb[dy * 64:(dy + 1) * 64],
                            in_=imgd[s, :, r0 + dy:r0 + 2 * R2:2, :])
                    pt = ptp.tile([128, R2, 258], F32R, tag='pt', name='pt')
                    nc.vector.tensor_copy(
                        out=pt[:, :, 256:258],
                        in_=ones2[:, None, :].to_broadcast((128, R2, 2)))
                    for t in range(R2):
                        for dx in range(2):
                            tp = xp.tile([128, 4, 128], F32, tag='xp',
                                         name='tpp')
                            nc.tensor.transpose(
                                tp[:, 0, :], img_sb[:, t, dx::2], ident[:])
                            dst = pt[:, t, dx * 128:(dx + 1) * 128]
                            if dx == 0:
                                nc.vector.tensor_copy(out=dst, in_=tp[:, 0, :])
                            else:
                                nc.scalar.activation(
                                    out=dst, in_=tp[:, 0, :],
                                    func=ACTF.Identity, bias=0.0, scale=1.0)
                        first = (ti == 0 and t == 0)
                        last = (ti == NT - 1 and t == R2 - 1)
                        for rc in range(2):
                            nc.tensor.matmul(
                                cpsums[rc][:],
                                pt[:, t, rc * 128:(rc + 1) * 128],
                                pt[:, t, 0:258],
                                start=first, stop=last)
                return cpsums

            # =============== tiny per-sample math ===============
            def tiny(s, cpsums):
                c_sb = work.tile([128, 2, 258], F32, tag='csb', name='csb')
                for rc in range(2):
                    nc.vector.tensor_copy(out=c_sb[:, rc, :], in_=cpsums[rc][:])

                a_sb = {}
                for br, c in BRANCHES:
                    chs = _chunks(c)
                    mc = len(chs)
                    wqt, wkt = cget(f'wqt_{br}'), cget(f'wkt_{br}')
                    vq_sb = work.tile([128, 2, c], F32, tag=f'vq_{br}',
                                      name='vq')
                    vk_sb = work.tile([128, 2, c], F32, tag=f'vk_{br}',
                                      name='vk')
                    for (vsb, wt) in ((vq_sb, wqt), (vk_sb, wkt)):
                        for rc in range(2):
                            vp = tps.tile([128, 258], F32, tag='tps', bufs=2,
                                          name='vp')
                            for mi in range(2):
                                nc.tensor.matmul(
                                    vp[:, 0:c],
                                    c_sb[:, mi, rc * 128:(rc + 1) * 128],
                                    wt[:, mi, :],
                                    start=(mi == 0), stop=(mi == 1))
                            nc.vector.tensor_copy(out=vsb[:, rc, :],
                                                  in_=vp[:, 0:c])
                    dcol = {}
                    for (dtag, wt, vsb) in (('q', wqt, vq_sb),
                                            ('k', wkt, vk_sb)):
                        dc = work.tile([128, mc, 1], F32, tag=f'd{dtag}_{br}',
                                       name='dc')
                        for mi, (o, sz) in enumerate(chs):
                            gp = tps.tile([128, 258], F32, tag='tps', bufs=2,
                                          name='gqp')
                            for rc in range(2):
                                nc.tensor.matmul(
                                    gp[:sz, 0:c],
                                    wt[:, rc, o:o + sz],
                                    vsb[:, rc, :],
                                    start=(rc == 0), stop=(rc == 1))
                            tmp = work.tile([128, c], F32, tag=f'tmp_{br}',
                                            name='dtmp')
                            nc.vector.tensor_tensor(
                                out=tmp[:sz, :], in0=gp[:sz, 0:c],
                                in1=cget(f'eye_{br}')[:sz, mi, :],
                                op=ALU.mult)
                            nc.vector.tensor_reduce(
                                out=dc[:sz, mi, :], in_=tmp[:sz, :],
                                axis=AX.X, op=ALU.add)
                        dcol[dtag] = dc
                    rows = {}
                    for (dtag, wt) in (('q', wqt), ('k', wkt)):
                        rp = tps.tile([128, 258], F32, tag='tps', bufs=2,
                                      name='rp')
                        for rc in range(2):
                            nc.tensor.matmul(
                                rp[0:1, 0:c], c_sb[:, rc, 256:257],
                                wt[:, rc, :], start=(rc == 0), stop=(rc == 1))
                        rsb = work.tile([1, c], F32, tag=f'{dtag}row_{br}',
                                        name='rsb')
                        nc.vector.tensor_copy(out=rsb[:], in_=rp[0:1, 0:c])
                        rows[dtag] = rsb
                    cols = {}
                    for (dtag, wt) in (('q', wqt), ('k', wkt)):
                        ccol = work.tile([128, mc, 1], F32,
                                         tag=f'{dtag}col_{br}', name='ccol')
                        for mi, (o, sz) in enumerate(chs):
                            cp = tps.tile([128, 8], F32, tag='tps_s',
                                          name='cp')
                            for rc in range(2):
                                nc.tensor.matmul(
                                    cp[:sz, 0:1], wt[:, rc, o:o + sz],
                                    c_sb[:, rc, 256:257],
                                    start=(rc == 0), stop=(rc == 1))
                            nc.vector.tensor_copy(out=ccol[:sz, mi, :],
                                                  in_=cp[:sz, 0:1])
                        cols[dtag] = ccol
                    rq_sb = work.tile([128, mc, 1], F32, tag=f'rq_{br}',
                                      name='rq')
                    sk_sb = work.tile([128, mc, 1], F32, tag=f'sk_{br}',
                                      name='skc')
                    for mi, (o, sz) in enumerate(chs):
                        nc.vector.tensor_tensor(
                            out=rq_sb[:sz, mi, :], in0=cols['q'][:sz, mi, :],
                            in1=cget(f'qb2_{br}')[:sz, mi, :], op=ALU.mult)
                        nc.vector.tensor_add(
                            out=rq_sb[:sz, mi, :], in0=rq_sb[:sz, mi, :],
                            in1=dcol['q'][:sz, mi, :])
                        nc.vector.tensor_add(
                            out=rq_sb[:sz, mi, :], in0=rq_sb[:sz, mi, :],
                            in1=cget(f'qb2n_{br}')[:sz, mi, :])
                        nc.scalar.activation(
                            out=rq_sb[:sz, mi, :], in_=rq_sb[:sz, mi, :],
                            func=ACTF.Sqrt, bias=0.0, scale=1.0)
                        nc.vector.tensor_scalar_max(
                            rq_sb[:sz, mi, :], rq_sb[:sz, mi, :], EPS)
                        nc.vector.reciprocal(out=rq_sb[:sz, mi, :],
                                             in_=rq_sb[:sz, mi, :])
                        nc.vector.tensor_scalar_mul(
                            rq_sb[:sz, mi, :], rq_sb[:sz, mi, :],
                            float(temps[br]))
                        nc.vector.tensor_tensor(
                            out=sk_sb[:sz, mi, :], in0=cols['k'][:sz, mi, :],
                            in1=cget(f'kb2_{br}')[:sz, mi, :], op=ALU.mult)
                        nc.vector.tensor_add(
                            out=sk_sb[:sz, mi, :], in0=sk_sb[:sz, mi, :],
                            in1=dcol['k'][:sz, mi, :])
                        nc.vector.tensor_add(
                            out=sk_sb[:sz, mi, :], in0=sk_sb[:sz, mi, :],
                            in1=cget(f'kb2n_{br}')[:sz, mi, :])
                    skr = tps.tile([128, 258], F32, tag='tps', bufs=2,
                                   name='skr')
                    for mi, (o, sz) in enumerate(chs):
                        nc.tensor.matmul(
                            skr[0:1, 0:c], sk_sb[:sz, mi, :],
                            cget(f'eye_{br}')[:sz, mi, :],
                            start=(mi == 0), stop=(mi == mc - 1))
                    rk_sb = work.tile([1, c], F32, tag=f'rk_{br}', name='rk')
                    nc.scalar.activation(out=rk_sb[:], in_=skr[0:1, 0:c],
                                         func=ACTF.Sqrt, bias=0.0, scale=1.0)
                    nc.vector.tensor_scalar_max(rk_sb[:], rk_sb[:], EPS)
                    nc.vector.reciprocal(out=rk_sb[:], in_=rk_sb[:])
                    tmr = work.tile([1, c], F32, tag=f'tmr_{br}', name='tmr')
                    nc.vector.tensor_scalar_mul(
                        tmr[:], cget(f'kbrow_{br}')[:], float(NPIX))
                    nc.vector.tensor_add(out=tmr[:], in0=tmr[:],
                                         in1=rows['k'][:])
                    a_t = work.tile([128, mc, c], F32, tag=f'a_{br}',
                                    name='a_t')
                    if br == 'h':
                        nc.gpsimd.memset(a_t[64:128, 1, :], 0.0)
                    else:
                        nc.gpsimd.memset(a_t[64:128, 0, :], 0.0)
                    for mi, (o, sz) in enumerate(chs):
                        gp = tps.tile([128, 258], F32, tag='tps', bufs=2,
                                      name='gp')
                        for rc in range(2):
                            nc.tensor.matmul(
                                gp[:sz, 0:c], wqt[:, rc, o:o + sz],
                                vk_sb[:, rc, :], start=(rc == 0), stop=False)
                        nc.tensor.matmul(
                            gp[:sz, 0:c], rows['q'][:, o:o + sz],
                            cget(f'kbrow_{br}')[:], start=False, stop=False)
                        nc.tensor.matmul(
                            gp[:sz, 0:c], cget(f'qbrow_{br}')[:, o:o + sz],
                            tmr[:], start=False, stop=True)
                        rkp = tps.tile([128, 258], F32, tag='tps2', name='rkp')
                        nc.tensor.matmul(rkp[:sz, 0:c], ones_row[:, 0:sz],
                                         rk_sb[:], start=True, stop=True)
                        s_t = work.tile([128, c], F32, tag=f's_{br}',
                                        name='s_t')
                        nc.vector.tensor_scalar_mul(
                            s_t[:sz, :], gp[:sz, 0:c], rq_sb[:sz, mi, :])
                        nc.vector.tensor_tensor(
                            out=s_t[:sz, :], in0=s_t[:sz, :],
                            in1=rkp[:sz, 0:c], op=ALU.mult)
                        nmax = work.tile([128, 1], F32, tag=f'nm_{br}',
                                         name='nmax')
                        nc.vector.tensor_reduce(
                            out=nmax[:sz], in_=s_t[:sz, :], axis=AX.X,
                            op=ALU.max, negate=True)
                        nc.scalar.activation(
                            out=a_t[:sz, mi, :], in_=s_t[:sz, :],
                            func=ACTF.Exp, bias=nmax[:sz], scale=1.0)
                        ssum = work.tile([128, 1], F32, tag=f'ss_{br}',
                                         name='ssum')
                        nc.vector.tensor_reduce(
                            out=ssum[:sz], in_=a_t[:sz, mi, :], axis=AX.X,
                            op=ALU.add)
                        nc.vector.reciprocal(out=ssum[:sz], in_=ssum[:sz])
                        nc.vector.tensor_scalar_mul(
                            a_t[:sz, mi, :], a_t[:sz, mi, :], ssum[:sz])
                    a_sb[br] = a_t

                t2 = {}
                for br, c in BRANCHES:
                    chs = _chunks(c)
                    mc = len(chs)
                    t2_sb = work.tile([128, mc, PC], F32, tag=f't2_{br}',
                                      name='t2sb')
                    for ji, (jo, jsz) in enumerate(chs):
                        if jsz < 128:
                            nc.gpsimd.memset(t2_sb[jsz:128, ji, :], 0.0)
                        tp2 = tps.tile([128, 258], F32, tag='tps', bufs=2,
                                       name='tp2')
                        for mi in range(mc):
                            nc.tensor.matmul(
                                tp2[:jsz, 0:PC],
                                a_sb[br][:, mi, jo:jo + jsz],
                                cget(f'pst_{br}')[:, mi, :],
                                start=(mi == 0), stop=(mi == mc - 1))
                        nc.vector.tensor_copy(out=t2_sb[:jsz, ji, :],
                                              in_=tp2[:jsz, 0:PC])
                    t2[br] = t2_sb

                w_t = ptp.tile([128, 2, PC], F32R, tag='wsb', name='wsb')
                seq = [(br, ji) for br, c in BRANCHES
                       for ji in range(len(_chunks(c)))]
                for nch in range(2):
                    wp = tps.tile([128, 258], F32, tag='tps', bufs=2,
                                  name='wp')
                    for i, (br, ji) in enumerate(seq):
                        nc.tensor.matmul(
                            wp[:, 0:PC],
                            cget(f'van_{br}')[:, ji, nch * 128:(nch + 1) * 128],
                            t2[br][:, ji, :],
                            start=(i == 0), stop=(i == len(seq) - 1))
                    nc.vector.tensor_copy(out=w_t[:, nch, :], in_=wp[:, 0:PC])

                pvs = {}
                for br, c in BRANCHES:
                    chs = _chunks(c)
                    pv = work.tile([128, len(chs), 1], F32, tag=f'pv_{br}',
                                   name='pv')
                    for mi, (o, sz) in enumerate(chs):
                        tmp = work.tile([128, c], F32, tag=f'tmp_{br}',
                                        name='pvt')
                        nc.vector.tensor_tensor(
                            out=tmp[:, :], in0=a_sb[br][:, mi, :],
                            in1=cget(f'vbf_{br}')[:, mi, :], op=ALU.mult)
                        nc.vector.tensor_reduce(
                            out=pv[:, mi, :], in_=tmp[:, :], axis=AX.X,
                            op=ALU.add)
                    pvs[br] = pv
                b_t = work.tile([128, 2, 1], F32, tag='bcol', name='bcol')
                seqb = [(br, mi) for br, c in BRANCHES
                        for mi in range(len(_chunks(c)))]
                for nch in range(2):
                    bp = tps.tile([128, 8], F32, tag='tps_s', name='bp')
                    for i, (br, mi) in enumerate(seqb):
                        nc.tensor.matmul(
                            bp[:, 0:1],
                            cget(f'pst_{br}')[:, mi, nch * 128:(nch + 1) * 128],
                            pvs[br][:, mi, :],
                            start=(i == 0), stop=(i == len(seqb) - 1))
                    nc.vector.tensor_tensor(
                        out=b_t[:, nch, :], in0=bp[:, 0:1],
                        in1=cget('bconst')[:, nch, :], op=ALU.add)
                return w_t, b_t

            # =============== PASS 2: apply W_big^T ===============
            def pass2(s):
                w_t, b_t = w_sb[s], bcol_sb[s]
                for ti in range(NT):
                    r0 = 2 * R2 * ti
                    img_sb = io.tile([128, R2, 256], F32, tag='in2',
                                     name='img2')
                    for dy in range(2):
                        nc.sync.dma_start(
                            out=img_sb[dy * 64:(dy + 1) * 64],
                            in_=imgd[s, :, r0 + dy:r0 + 2 * R2:2, :])
                    x2 = ptp.tile([128, 2, R2, 128], F32R, tag='x2',
                                  name='x2')
                    nc.vector.tensor_copy(out=x2[:, 0], in_=img_sb[:, :, 0::2])
                    nc.scalar.activation(out=x2[:, 1], in_=img_sb[:, :, 1::2],
                                         func=ACTF.Identity, bias=0.0,
                                         scale=1.0)
                    out_sb = io.tile([128, R2, 256], F32, tag='out',
                                     name='outsb')
                    for sub in range(R2 // 4):
                        for nch in range(2):
                            op = xp.tile([128, 4, 128], F32, tag='xp',
                                         name='op')
                            for dx in range(2):
                                nc.tensor.matmul(
                                    op[:, :, :],
                                    w_t[:, dx, nch * 128:(nch + 1) * 128],
                                    x2[:, dx, sub * 4:(sub + 1) * 4, :],
                                    start=(dx == 0), stop=(dx == 1))
                            nc.scalar.activation(
                                out=out_sb[:, sub * 4:(sub + 1) * 4, nch::2],
                                in_=op[:, :, :],
                                func=ACTF.Identity,
                                bias=b_t[:, nch, :], scale=1.0)
                    for dy in range(2):
                        nc.scalar.dma_start(
                            out=outd[s, :, r0 + dy:r0 + 2 * R2:2, :],
                            in_=out_sb[dy * 64:(dy + 1) * 64])

            # =============== schedule ===============
            cp0 = pass1(0)
            w_sb[0], bcol_sb[0] = tiny(0, cp0)
            cp1 = pass1(1)
            pass2(0)
            w_sb[1], bcol_sb[1] = tiny(1, cp1)
            pass2(1)

    nc.finalize()
    return nc


_CACHE = {}


def get_program(temps):
    key = tuple(sorted(temps.items()))
    if key not in _CACHE:
        _CACHE[key] = build_program(temps)
    return _CACHE[key]


def make_in_maps(inputs):
    inputs = {k: np.asarray(v) for k, v in inputs.items()}
    consts, temps = build_host_consts(inputs)
    img = np.ascontiguousarray(inputs['img'], dtype=np.float32)
    in_maps = []
    for core in range(NCORES):
        m = {'img': img[core * SPC:(core + 1) * SPC]}
        m.update(consts)
        in_maps.append(m)
    return in_maps, temps


def kernel(**inputs):
    in_maps, temps = make_in_maps(inputs)
    nc = get_program(temps)
    from concourse.bass_utils import run_bass_kernel_spmd
    res = run_bass_kernel_spmd(nc, in_maps, core_ids=list(range(NCORES)),
                               trace=False)
    out = np.concatenate([res.results[c]['out'] for c in range(NCORES)],
                         axis=0)
    return out.astype(np.float32)


# revision 9
# speedup vs baseline: 534.5628x; 534.5628x over previous
"""Trainium2 Bass kernel for nn_DWTenhance (Haar DWT + dual MDTA + inverse DWT).

Exact algorithmic reformulation:
  Per sample the whole network is an affine map of the 2x2-polyphase
  representation P (256 x n, n = 128*128) of the image:
      out_polyphase = W_big @ P + b_big,
  where W_big = SY @ blockdiag(pw_ll A_ll (vw_ll@AN_ll), pw_h A_h (vw_h@AN_h))
  (AN/SY = Haar analysis/synthesis as 256x256 channel mixers) and the
  attention matrices A depend on the data only through the 256x256 Gram
  matrix C = P@P^T and the row sums s = P@1:
      q k^T = Wq~ C Wk~^T + (Wq~ s) kb^T + qb (Wk~ s)^T + n qb kb^T,
      |q_i|^2 = (Wq~ C Wq~^T)_ii + 2 qb_i (Wq~ s)_i + n qb_i^2, etc.

  Pass 1: stream image, PE-transpose 128x128 tiles to pixel-major layout,
          accumulate C (+ s via two ones-columns) in PSUM with f32r matmuls.
  Tiny:   on-chip small-matrix algebra + row softmax -> W_big^T (f32r) and
          bias column per sample.
  Pass 2: stream image again, apply W_big^T as a 256->256 block conv on the
          polyphase channels, add bias, interleave back, store.

Data parallel over batch: 16 samples / 8 NeuronCores = 2 samples per core.
"""
import sys

sys.path.insert(0, '/opt/trn_rl_repo')

import numpy as np

import concourse.bass as bass
import concourse.tile as tile
from concourse import bacc, mybir
from concourse.masks import make_identity

F32 = mybir.dt.float32
F32R = mybir.dt.float32r
AX = mybir.AxisListType
ALU = mybir.AluOpType
ACTF = mybir.ActivationFunctionType

B, C, H, W = 16, 64, 256, 256
NCORES = 8
SPC = B // NCORES           # samples per core
H2, W2 = H // 2, W // 2
NPIX = H2 * W2              # 16384
PC = 4 * C                  # 256 polyphase channels
R2 = 8                      # half-res rows per streamed tile
NT = H2 // R2               # tiles per sample
EPS = 1e-12

BRANCHES = (('l', C), ('h', 3 * C))


def _pidx(dx, dy, ci):
    # polyphase channel order; must match the pass-1 transpose layout
    return dx * 2 * C + dy * C + ci


def _build_AN_SY():
    AN = np.zeros((PC, PC))
    SY = np.zeros((PC, PC))
    for ci in range(C):
        a, b, c_, d = (_pidx(0, 0, ci), _pidx(1, 0, ci),
                       _pidx(0, 1, ci), _pidx(1, 1, ci))
        AN[ci, [a, b, c_, d]] = [0.5, 0.5, 0.5, 0.5]
        AN[C + 3 * ci + 0, [a, b, c_, d]] = [0.5, -0.5, 0.5, -0.5]
        AN[C + 3 * ci + 1, [a, b, c_, d]] = [0.5, 0.5, -0.5, -0.5]
        AN[C + 3 * ci + 2, [a, b, c_, d]] = [0.5, -0.5, -0.5, 0.5]
        l, h1, h2, h3 = ci, C + 3 * ci, C + 3 * ci + 1, C + 3 * ci + 2
        SY[a, [l, h1, h2, h3]] = [0.5, 0.5, 0.5, 0.5]
        SY[b, [l, h1, h2, h3]] = [0.5, -0.5, 0.5, -0.5]
        SY[c_, [l, h1, h2, h3]] = [0.5, 0.5, -0.5, -0.5]
        SY[d, [l, h1, h2, h3]] = [0.5, -0.5, -0.5, 0.5]
    return AN, SY


def _chunks(c):
    out = []
    i = 0
    while i < c:
        out.append((i, min(128, c - i)))
        i += 128
    return out


def build_host_consts(inputs):
    """float64 host preprocessing of the small weights -> DRAM const arrays."""
    AN, SY = _build_AN_SY()
    f = np.float64
    consts = {}
    temps = {}
    bias_vec = np.zeros(PC)
    for br, key, c in (('l', 'll', C), ('h', 'h', 3 * C)):
        qw, qb = inputs[f'{key}_qw'].astype(f), inputs[f'{key}_qb'].astype(f)
        kw, kb = inputs[f'{key}_kw'].astype(f), inputs[f'{key}_kb'].astype(f)
        vw, vb = inputs[f'{key}_vw'].astype(f), inputs[f'{key}_vb'].astype(f)
        pw, pb = inputs[f'{key}_pw'].astype(f), inputs[f'{key}_pb'].astype(f)
        temps[br] = float(np.asarray(inputs[f'{key}_temp']).reshape(-1)[0])
        ANb = AN[:C] if br == 'l' else AN[C:]
        SYb = SY[:, :C] if br == 'l' else SY[:, C:]
        Wq = qw @ ANb          # (c, 256)
        Wk = kw @ ANb
        VAN = vw @ ANb         # (c, 256)
        PS = SYb @ pw          # (256, c)
        chs = _chunks(c)
        mc = len(chs)
        consts[f'wqt_{br}'] = np.ascontiguousarray(
            Wq.T.reshape(2, 128, c)).astype(np.float32)
        consts[f'wkt_{br}'] = np.ascontiguousarray(
            Wk.T.reshape(2, 128, c)).astype(np.float32)
        van = np.zeros((mc, 128, PC))
        pst = np.zeros((mc, 128, PC))
        eye = np.zeros((mc, 128, c))
        vbf = np.zeros((mc, 128, c))
        for mi, (o, sz) in enumerate(chs):
            van[mi, :sz] = VAN[o:o + sz]
            pst[mi, :sz] = PS.T[o:o + sz]
            eye[mi, :sz, o:o + sz] = np.eye(sz)
            vbf[mi, :, :] = vb[None, :]
        consts[f'van_{br}'] = van.astype(np.float32)
        consts[f'pst_{br}'] = pst.astype(np.float32)
        consts[f'eye_{br}'] = eye.astype(np.float32)
        consts[f'vbf_{br}'] = vbf.astype(np.float32)

        def colchunks(v):
            arr = np.zeros((mc, 128, 1))
            for mi, (o, sz) in enumerate(chs):
                arr[mi, :sz, 0] = v[o:o + sz]
            return arr.astype(np.float32)
        consts[f'qb2_{br}'] = colchunks(2.0 * qb)
        consts[f'qb2n_{br}'] = colchunks(NPIX * qb * qb)
        consts[f'kb2_{br}'] = colchunks(2.0 * kb)
        consts[f'kb2n_{br}'] = colchunks(NPIX * kb * kb)
        consts[f'qbrow_{br}'] = qb[None, :].astype(np.float32)
        consts[f'kbrow_{br}'] = kb[None, :].astype(np.float32)
        bias_vec += SYb @ pb
    consts['bconst'] = np.ascontiguousarray(
        bias_vec.reshape(2, 128, 1)).astype(np.float32)
    return consts, temps


CSHAPES = {
    'wqt_l': [2, 128, C], 'wkt_l': [2, 128, C],
    'wqt_h': [2, 128, 3 * C], 'wkt_h': [2, 128, 3 * C],
    'van_l': [1, 128, PC], 'pst_l': [1, 128, PC],
    'van_h': [2, 128, PC], 'pst_h': [2, 128, PC],
    'eye_l': [1, 128, C], 'vbf_l': [1, 128, C],
    'eye_h': [2, 128, 3 * C], 'vbf_h': [2, 128, 3 * C],
    'qb2_l': [1, 128, 1], 'qb2n_l': [1, 128, 1],
    'kb2_l': [1, 128, 1], 'kb2n_l': [1, 128, 1],
    'qb2_h': [2, 128, 1], 'qb2n_h': [2, 128, 1],
    'kb2_h': [2, 128, 1], 'kb2n_h': [2, 128, 1],
    'qbrow_l': [1, C], 'kbrow_l': [1, C],
    'qbrow_h': [1, 3 * C], 'kbrow_h': [1, 3 * C],
    'bconst': [2, 128, 1],
}


DEFAULT_CFG = dict(io_bufs=3, ptp_bufs=3, xp_bufs=2, xp2_bufs=2,
                   tps_bufs=1, p1dt='f32r', p2dt='f32r',
                   phases='A0 B0 A1 C0 B1 C1', loops=1)


def build_program(temps, cfg=None):
    """Build the Bacc program one core runs (SPC samples, full pipeline)."""
    cfg = dict(DEFAULT_CFG, **(cfg or {}))
    DT = {'f32r': F32R, 'bf16': mybir.dt.bfloat16, 'f32': F32}
    P1DT = DT[cfg['p1dt']]
    P2DT = DT[cfg['p2dt']]
    nc = bacc.Bacc()

    imgd = nc.declare_dram_parameter('img', [SPC, C, H, W], F32, isOutput=False)
    outd = nc.declare_dram_parameter('out', [SPC, C, H, W], F32, isOutput=True)
    cdecl = {n: nc.declare_dram_parameter(n, s, F32, isOutput=False)
             for n, s in CSHAPES.items()}

    with tile.TileContext(nc) as tc:
        with tc.tile_pool(name='cst', bufs=1) as cst, \
             tc.tile_pool(name='io', bufs=cfg['io_bufs']) as io, \
             tc.tile_pool(name='ptp', bufs=cfg['ptp_bufs']) as ptp, \
             tc.tile_pool(name='work', bufs=2) as work, \
             tc.tile_pool(name='xp', bufs=cfg['xp_bufs'], space='PSUM') as xp, \
             tc.tile_pool(name='xp2', bufs=cfg['xp2_bufs'], space='PSUM') as xp2, \
             tc.tile_pool(name='cps', bufs=1, space='PSUM') as cps, \
             tc.tile_pool(name='tps', bufs=1, space='PSUM') as tps:

            # ---------------- constants ----------------
            ident = cst.tile([128, 128], F32)
            make_identity(nc, ident)
            ones2 = cst.tile([128, 2], F32)
            nc.vector.memset(ones2, 1.0)
            ones_row = cst.tile([1, 128], F32)
            nc.vector.memset(ones_row, 1.0)
            csb = {}
            for name, shp in CSHAPES.items():
                if len(shp) == 3:
                    t = cst.tile([128, shp[0], shp[2]], F32, tag=name)
                    nc.sync.dma_start(
                        out=t,
                        in_=cdecl[name][:, :, :].rearrange('c p x -> p c x'))
                else:
                    t = cst.tile([1, shp[1]], F32, tag=name)
                    nc.sync.dma_start(out=t, in_=cdecl[name][:, :])
                csb[name] = t

            CFG_TPS = cfg['tps_bufs']

            def cget(name):
                return csb[name]

            w_sb = {}
            bcol_sb = {}

            # =============== PASS 1: Gram accumulation ===============
            def pass1_alloc(s):
                return [cps.tile([128, 258], F32, tag=f'c{rc}', name=f'cp{rc}')
                        for rc in range(2)]

            def pass1_tile(s, ti, cpsums):
                if True:
                    r0 = 2 * R2 * ti
                    img_sb = io.tile([128, R2, 256], F32, tag='in1',
                                     name='img1')
                    for dy in range(2):
                        nc.sync.dma_start(
                            out=img_sb[dy * 64:(dy + 1) * 64],
                            in_=imgd[s, :, r0 + dy:r0 + 2 * R2:2, :])
                    pt = ptp.tile([128, R2, 258], P1DT, tag='pt', name='pt')
                    nc.vector.tensor_copy(
                        out=pt[:, :, 256:258],
                        in_=ones2[:, None, :].to_broadcast((128, R2, 2)))
                    for tp_i in range(R2 // 2):
                        tpp = xp.tile([128, 2, 256], F32, tag='xpa',
                                      name='tpp')
                        for tt in range(2):
                            t = 2 * tp_i + tt
                            for dx in range(2):
                                nc.tensor.transpose(
                                    tpp[:, tt, dx * 128:(dx + 1) * 128],
                                    img_sb[:, t, dx::2], ident[:])
                        dst = pt[:, 2 * tp_i:2 * tp_i + 2, 0:256]
                        if tp_i % 2 == 0:
                            nc.vector.tensor_copy(out=dst, in_=tpp[:, :, :])
                        else:
                            nc.scalar.activation(
                                out=dst, in_=tpp[:, :, :],
                                func=ACTF.Identity, bias=0.0, scale=1.0)
                        for tt in range(2):
                            t = 2 * tp_i + tt
                            first = (ti == 0 and t == 0)
                            last = (ti == NT - 1 and t == R2 - 1)
                            for rc in range(2):
                                nc.tensor.matmul(
                                    cpsums[rc][:],
                                    pt[:, t, rc * 128:(rc + 1) * 128],
                                    pt[:, t, 0:258],
                                    start=first, stop=last)

            def pass1(s):
                cpsums = pass1_alloc(s)
                for ti in range(NT):
                    pass1_tile(s, ti, cpsums)
                return cpsums

            # =============== tiny per-sample math ===============
            def tiny(s, cpsums):
                c_sb = work.tile([128, 2, 258], F32, tag='csb', name='csb')
                for rc in range(2):
                    nc.vector.tensor_copy(out=c_sb[:, rc, :], in_=cpsums[rc][:])

                a_sb = {}
                for br, c in BRANCHES:
                    chs = _chunks(c)
                    mc = len(chs)
                    wqt, wkt = cget(f'wqt_{br}'), cget(f'wkt_{br}')
                    vq_sb = work.tile([128, 2, c], F32, tag=f'vq_{br}',
                                      name='vq')
                    vk_sb = work.tile([128, 2, c], F32, tag=f'vk_{br}',
                                      name='vk')
                    for (vsb, wt) in ((vq_sb, wqt), (vk_sb, wkt)):
                        for rc in range(2):
                            vp = tps.tile([128, 258], F32, tag='tps', bufs=CFG_TPS,
                                          name='vp')
                            for mi in range(2):
                                nc.tensor.matmul(
                                    vp[:, 0:c],
                                    c_sb[:, mi, rc * 128:(rc + 1) * 128],
                                    wt[:, mi, :],
                                    start=(mi == 0), stop=(mi == 1))
                            nc.vector.tensor_copy(out=vsb[:, rc, :],
                                                  in_=vp[:, 0:c])
                    dcol = {}
                    for (dtag, wt, vsb) in (('q', wqt, vq_sb),
                                            ('k', wkt, vk_sb)):
                        dc = work.tile([128, mc, 1], F32, tag=f'd{dtag}_{br}',
                                       name='dc')
                        for mi, (o, sz) in enumerate(chs):
                            gp = tps.tile([128, 258], F32, tag='tps', bufs=CFG_TPS,
                                          name='gqp')
                            for rc in range(2):
                                nc.tensor.matmul(
                                    gp[:sz, 0:c],
                                    wt[:, rc, o:o + sz],
                                    vsb[:, rc, :],
                                    start=(rc == 0), stop=(rc == 1))
                            tmp = work.tile([128, c], F32, tag=f'tmp_{br}',
                                            name='dtmp')
                            nc.vector.tensor_tensor(
                                out=tmp[:sz, :], in0=gp[:sz, 0:c],
                                in1=cget(f'eye_{br}')[:sz, mi, :],
                                op=ALU.mult)
                            nc.vector.tensor_reduce(
                                out=dc[:sz, mi, :], in_=tmp[:sz, :],
                                axis=AX.X, op=ALU.add)
                        dcol[dtag] = dc
                    rows = {}
                    for (dtag, wt) in (('q', wqt), ('k', wkt)):
                        rp = tps.tile([128, 258], F32, tag='tps', bufs=CFG_TPS,
                                      name='rp')
                        for rc in range(2):
                            nc.tensor.matmul(
                                rp[0:1, 0:c], c_sb[:, rc, 256:257],
                                wt[:, rc, :], start=(rc == 0), stop=(rc == 1))
                        rsb = work.tile([1, c], F32, tag=f'{dtag}row_{br}',
                                        name='rsb')
                        nc.vector.tensor_copy(out=rsb[:], in_=rp[0:1, 0:c])
                        rows[dtag] = rsb
                    cols = {}
                    for (dtag, wt) in (('q', wqt), ('k', wkt)):
                        ccol = work.tile([128, mc, 1], F32,
                                         tag=f'{dtag}col_{br}', name='ccol')
                        for mi, (o, sz) in enumerate(chs):
                            cp = tps.tile([128, 8], F32, tag='tps2',
                                          name='cp')
                            for rc in range(2):
                                nc.tensor.matmul(
                                    cp[:sz, 0:1], wt[:, rc, o:o + sz],
                                    c_sb[:, rc, 256:257],
                                    start=(rc == 0), stop=(rc == 1))
                            nc.vector.tensor_copy(out=ccol[:sz, mi, :],
                                                  in_=cp[:sz, 0:1])
                        cols[dtag] = ccol
                    rq_sb = work.tile([128, mc, 1], F32, tag=f'rq_{br}',
                                      name='rq')
                    sk_sb = work.tile([128, mc, 1], F32, tag=f'sk_{br}',
                                      name='skc')
                    for mi, (o, sz) in enumerate(chs):
                        nc.vector.tensor_tensor(
                            out=rq_sb[:sz, mi, :], in0=cols['q'][:sz, mi, :],
                            in1=cget(f'qb2_{br}')[:sz, mi, :], op=ALU.mult)
                        nc.vector.tensor_add(
                            out=rq_sb[:sz, mi, :], in0=rq_sb[:sz, mi, :],
                            in1=dcol['q'][:sz, mi, :])
                        nc.vector.tensor_add(
                            out=rq_sb[:sz, mi, :], in0=rq_sb[:sz, mi, :],
                            in1=cget(f'qb2n_{br}')[:sz, mi, :])
                        nc.scalar.activation(
                            out=rq_sb[:sz, mi, :], in_=rq_sb[:sz, mi, :],
                            func=ACTF.Sqrt, bias=0.0, scale=1.0)
                        nc.vector.tensor_scalar_max(
                            rq_sb[:sz, mi, :], rq_sb[:sz, mi, :], EPS)
                        nc.vector.reciprocal(out=rq_sb[:sz, mi, :],
                                             in_=rq_sb[:sz, mi, :])
                        nc.vector.tensor_scalar_mul(
                            rq_sb[:sz, mi, :], rq_sb[:sz, mi, :],
                            float(temps[br]))
                        nc.vector.tensor_tensor(
                            out=sk_sb[:sz, mi, :], in0=cols['k'][:sz, mi, :],
                            in1=cget(f'kb2_{br}')[:sz, mi, :], op=ALU.mult)
                        nc.vector.tensor_add(
                            out=sk_sb[:sz, mi, :], in0=sk_sb[:sz, mi, :],
                            in1=dcol['k'][:sz, mi, :])
                        nc.vector.tensor_add(
                            out=sk_sb[:sz, mi, :], in0=sk_sb[:sz, mi, :],
                            in1=cget(f'kb2n_{br}')[:sz, mi, :])
                    skr = tps.tile([128, 258], F32, tag='tps', bufs=CFG_TPS,
                                   name='skr')
                    for mi, (o, sz) in enumerate(chs):
                        nc.tensor.matmul(
                            skr[0:1, 0:c], sk_sb[:sz, mi, :],
                            cget(f'eye_{br}')[:sz, mi, :],
                            start=(mi == 0), stop=(mi == mc - 1))
                    rk_sb = work.tile([1, c], F32, tag=f'rk_{br}', name='rk')
                    nc.scalar.activation(out=rk_sb[:], in_=skr[0:1, 0:c],
                                         func=ACTF.Sqrt, bias=0.0, scale=1.0)
                    nc.vector.tensor_scalar_max(rk_sb[:], rk_sb[:], EPS)
                    nc.vector.reciprocal(out=rk_sb[:], in_=rk_sb[:])
                    tmr = work.tile([1, c], F32, tag=f'tmr_{br}', name='tmr')
                    nc.vector.tensor_scalar_mul(
                        tmr[:], cget(f'kbrow_{br}')[:], float(NPIX))
                    nc.vector.tensor_add(out=tmr[:], in0=tmr[:],
                                         in1=rows['k'][:])
                    a_t = work.tile([128, mc, c], F32, tag=f'a_{br}',
                                    name='a_t')
                    if br == 'h':
                        nc.gpsimd.memset(a_t[64:128, 1, :], 0.0)
                    else:
                        nc.gpsimd.memset(a_t[64:128, 0, :], 0.0)
                    for mi, (o, sz) in enumerate(chs):
                        gp = tps.tile([128, 258], F32, tag='tps', bufs=CFG_TPS,
                                      name='gp')
                        for rc in range(2):
                            nc.tensor.matmul(
                                gp[:sz, 0:c], wqt[:, rc, o:o + sz],
                                vk_sb[:, rc, :], start=(rc == 0), stop=False)
                        nc.tensor.matmul(
                            gp[:sz, 0:c], rows['q'][:, o:o + sz],
                            cget(f'kbrow_{br}')[:], start=False, stop=False)
                        nc.tensor.matmul(
                            gp[:sz, 0:c], cget(f'qbrow_{br}')[:, o:o + sz],
                            tmr[:], start=False, stop=True)
                        rkp = tps.tile([128, 258], F32, tag='tps2', name='rkp')
                        nc.tensor.matmul(rkp[:sz, 0:c], ones_row[:, 0:sz],
                                         rk_sb[:], start=True, stop=True)
                        s_t = work.tile([128, c], F32, tag=f's_{br}',
                                        name='s_t')
                        nc.vector.tensor_scalar_mul(
                            s_t[:sz, :], gp[:sz, 0:c], rq_sb[:sz, mi, :])
                        nc.vector.tensor_tensor(
                            out=s_t[:sz, :], in0=s_t[:sz, :],
                            in1=rkp[:sz, 0:c], op=ALU.mult)
                        nmax = work.tile([128, 1], F32, tag=f'nm_{br}',
                                         name='nmax')
                        nc.vector.tensor_reduce(
                            out=nmax[:sz], in_=s_t[:sz, :], axis=AX.X,
                            op=ALU.max, negate=True)
                        nc.scalar.activation(
                            out=a_t[:sz, mi, :], in_=s_t[:sz, :],
                            func=ACTF.Exp, bias=nmax[:sz], scale=1.0)
                        ssum = work.tile([128, 1], F32, tag=f'ss_{br}',
                                         name='ssum')
                        nc.vector.tensor_reduce(
                            out=ssum[:sz], in_=a_t[:sz, mi, :], axis=AX.X,
                            op=ALU.add)
                        nc.vector.reciprocal(out=ssum[:sz], in_=ssum[:sz])
                        nc.vector.tensor_scalar_mul(
                            a_t[:sz, mi, :], a_t[:sz, mi, :], ssum[:sz])
                    a_sb[br] = a_t

                t2 = {}
                for br, c in BRANCHES:
                    chs = _chunks(c)
                    mc = len(chs)
                    t2_sb = work.tile([128, mc, PC], F32, tag=f't2_{br}',
                                      name='t2sb')
                    for ji, (jo, jsz) in enumerate(chs):
                        if jsz < 128:
                            nc.gpsimd.memset(t2_sb[jsz:128, ji, :], 0.0)
                        tp2 = tps.tile([128, 258], F32, tag='tps', bufs=CFG_TPS,
                                       name='tp2')
                        for mi in range(mc):
                            nc.tensor.matmul(
                                tp2[:jsz, 0:PC],
                                a_sb[br][:, mi, jo:jo + jsz],
                                cget(f'pst_{br}')[:, mi, :],
                                start=(mi == 0), stop=(mi == mc - 1))
                        nc.vector.tensor_copy(out=t2_sb[:jsz, ji, :],
                                              in_=tp2[:jsz, 0:PC])
                    t2[br] = t2_sb

                w_t = ptp.tile([128, 2, PC], P2DT, tag='wsb', name='wsb')
                seq = [(br, ji) for br, c in BRANCHES
                       for ji in range(len(_chunks(c)))]
                for nch in range(2):
                    wp = tps.tile([128, 258], F32, tag='tps', bufs=CFG_TPS,
                                  name='wp')
                    for i, (br, ji) in enumerate(seq):
                        nc.tensor.matmul(
                            wp[:, 0:PC],
                            cget(f'van_{br}')[:, ji, nch * 128:(nch + 1) * 128],
                            t2[br][:, ji, :],
                            start=(i == 0), stop=(i == len(seq) - 1))
                    nc.vector.tensor_copy(out=w_t[:, nch, :], in_=wp[:, 0:PC])

                pvs = {}
                for br, c in BRANCHES:
                    chs = _chunks(c)
                    pv = work.tile([128, len(chs), 1], F32, tag=f'pv_{br}',
                                   name='pv')
                    for mi, (o, sz) in enumerate(chs):
                        tmp = work.tile([128, c], F32, tag=f'tmp_{br}',
                                        name='pvt')
                        nc.vector.tensor_tensor(
                            out=tmp[:, :], in0=a_sb[br][:, mi, :],
                            in1=cget(f'vbf_{br}')[:, mi, :], op=ALU.mult)
                        nc.vector.tensor_reduce(
                            out=pv[:, mi, :], in_=tmp[:, :], axis=AX.X,
                            op=ALU.add)
                    pvs[br] = pv
                b_t = work.tile([128, 2, 1], F32, tag='bcol', name='bcol')
                seqb = [(br, mi) for br, c in BRANCHES
                        for mi in range(len(_chunks(c)))]
                for nch in range(2):
                    bp = tps.tile([128, 8], F32, tag='tps2', name='bp')
                    for i, (br, mi) in enumerate(seqb):
                        nc.tensor.matmul(
                            bp[:, 0:1],
                            cget(f'pst_{br}')[:, mi, nch * 128:(nch + 1) * 128],
                            pvs[br][:, mi, :],
                            start=(i == 0), stop=(i == len(seqb) - 1))
                    nc.vector.tensor_tensor(
                        out=b_t[:, nch, :], in0=bp[:, 0:1],
                        in1=cget('bconst')[:, nch, :], op=ALU.add)
                return w_t, b_t

            # =============== PASS 2: apply W_big^T ===============
            def pass2_tile(s, ti):
                w_t, b_t = w_sb[s], bcol_sb[s]
                if True:
                    r0 = 2 * R2 * ti
                    img_sb = io.tile([128, R2, 256], F32, tag='in2',
                                     name='img2')
                    for dy in range(2):
                        nc.sync.dma_start(
                            out=img_sb[dy * 64:(dy + 1) * 64],
                            in_=imgd[s, :, r0 + dy:r0 + 2 * R2:2, :])
                    x2 = ptp.tile([128, 2, R2, 128], P2DT, tag='x2',
                                  name='x2')
                    nc.vector.tensor_copy(out=x2[:, 0], in_=img_sb[:, :, 0::2])
                    nc.scalar.activation(out=x2[:, 1], in_=img_sb[:, :, 1::2],
                                         func=ACTF.Identity, bias=0.0,
                                         scale=1.0)
                    out_sb = io.tile([128, R2, 256], F32, tag='out',
                                     name='outsb')
                    for sub in range(R2 // 4):
                        for nch in range(2):
                            op = xp2.tile([128, 4, 128], F32, tag='xpc',
                                          name='op')
                            for dx in range(2):
                                nc.tensor.matmul(
                                    op[:, :, :],
                                    w_t[:, dx, nch * 128:(nch + 1) * 128],
                                    x2[:, dx, sub * 4:(sub + 1) * 4, :],
                                    start=(dx == 0), stop=(dx == 1))
                            nc.scalar.activation(
                                out=out_sb[:, sub * 4:(sub + 1) * 4, nch::2],
                                in_=op[:, :, :],
                                func=ACTF.Identity,
                                bias=b_t[:, nch, :], scale=1.0)
                    for dy in range(2):
                        nc.scalar.dma_start(
                            out=outd[s, :, r0 + dy:r0 + 2 * R2:2, :],
                            in_=out_sb[dy * 64:(dy + 1) * 64])

            def pass2(s):
                for ti in range(NT):
                    pass2_tile(s, ti)

            # =============== schedule ===============
            def body(_i=None):
                cph = {}
                for ph in cfg['phases'].split():
                    kind, sidx = ph[0], int(ph[1])
                    if kind == 'A':
                        cph[sidx] = pass1(sidx)
                    elif kind == 'B':
                        w_sb[sidx], bcol_sb[sidx] = tiny(sidx, cph[sidx])
                    elif kind == 'C':
                        pass2(sidx)
                    elif kind == 'D':
                        for ti in range(NT):
                            r0 = 2 * R2 * ti
                            i1 = io.tile([128, R2, 256], F32, tag='in1',
                                         name='d1')
                            i2 = io.tile([128, R2, 256], F32, tag='in2',
                                         name='d2')
                            for dy in range(2):
                                nc.sync.dma_start(
                                    out=i1[dy * 64:(dy + 1) * 64],
                                    in_=imgd[sidx, :, r0 + dy:r0 + 2 * R2:2, :])
                                nc.sync.dma_start(
                                    out=i2[dy * 64:(dy + 1) * 64],
                                    in_=imgd[sidx, :, r0 + dy:r0 + 2 * R2:2, :])
                                nc.scalar.dma_start(
                                    out=outd[sidx, :, r0 + dy:r0 + 2 * R2:2, :],
                                    in_=i2[dy * 64:(dy + 1) * 64])
                    elif kind == 'M':
                        sb_ = int(ph[2])
                        cph[sb_] = pass1_alloc(sb_)
                        for ti in range(NT):
                            pass2_tile(sidx, ti)
                            pass1_tile(sb_, ti, cph[sb_])

            if cfg['loops'] == 1:
                body()
            else:
                with tc.For_i(0, cfg['loops'], 1) as _i:
                    body(_i)

    nc.finalize()
    return nc


_CACHE = {}


def get_program(temps, cfg=None):
    key = (tuple(sorted(temps.items())),
           tuple(sorted((cfg or {}).items(), key=str)))
    if key not in _CACHE:
        _CACHE[key] = build_program(temps, cfg)
    return _CACHE[key]


def make_in_maps(inputs):
    inputs = {k: np.asarray(v) for k, v in inputs.items()}
    consts, temps = build_host_consts(inputs)
    img = np.ascontiguousarray(inputs['img'], dtype=np.float32)
    in_maps = []
    for core in range(NCORES):
        m = {'img': img[core * SPC:(core + 1) * SPC]}
        m.update(consts)
        in_maps.append(m)
    return in_maps, temps


def kernel(**inputs):
    in_maps, temps = make_in_maps(inputs)
    nc = get_program(temps)
    from concourse.bass_utils import run_bass_kernel_spmd
    res = run_bass_kernel_spmd(nc, in_maps, core_ids=list(range(NCORES)),
                               trace=False)
    out = np.concatenate([res.results[c]['out'] for c in range(NCORES)],
                         axis=0)
    return out.astype(np.float32)
